# revision 1
# baseline (speedup 1.0000x reference)
"""HDClassifier Trainium2 kernel.

Math (per batch b):
  idx[t,c]   = clip(round((x+100)/200*200), 0, 200)
  bundled[t] = sum_c level_hv[idx[t,c]] * channel_hv[c]          # ints in [-8,8]
  u[t]       = roll(bundled[t],1) * bundled[t+1]                 # ints |.|<=64
  gram[t']   = roll(u[t'],2) * u[t'+2]                           # ints |.|<=4096
             (= prod_i roll(bundled[t'+i], 3-i), the 4-gram)
  sample     = sum_t' gram[t']                                   # exact in fp32
  out        = sign(sample) @ centroid.T

Device strategy (8 cores, 4 batches each):
  - Host folds channel_hv into the level table: M[c*201+l] = level_hv[l]*channel_hv[c],
    stored fp8(e4m3) (+-1 exact), padded to 1792 = 7*256 rows -> SBUF resident.
  - Host builds per-(b) one-hot planes onehot[k, t] = (k == c*201+idx[t,c]) in fp8.
    (quantization done host-side in exact fp32 to match jax bit-for-bit)
  - PE: bundled[t, d] = sum_k onehot[k,t] * M[k,d]  -- 7 DoubleRow k-passes of 256
    accumulated in PSUM (fp32, exact). ACT drains PSUM -> SBUF as fp8 (exact).
  - DVE: u = TT-mul (fp8 in, bf16 out, exact <=64); gram = TT-mul (bf16 in, f32 out).
  - PE: t'-sum via e_b ones-matmul (lhsT bf16 selection col, rhs f32 gram) -> PSUM
    [4, 500] accumulated over the 4 local batches -> DMA to DRAM.
  - Host: sign + tiny [32,10000]@[10000,6] matmul.
"""

import sys

sys.path.insert(0, "/opt/trn_rl_repo")

import numpy as np

import concourse.bass as bass
import concourse.mybir as mybir
from concourse import bacc
from concourse.bass_utils import run_bass_kernel_spmd
from concourse.tile import TileContext

# Problem constants (hardcoded per contract)
NUM_LEVELS = 201
N_GRAM = 4
B, T, C, D, NUM_CLASSES = 32, 128, 8, 10000, 6
N_CORES = 8
B_LOC = B // N_CORES  # 4 batches per core
K_TOT = C * NUM_LEVELS  # 1608
KT = 13  # k-tiles of 128 (non-DR path)
KP = 7  # DoubleRow k-passes of 256
K_PAD = KP * 256  # 1792
NCH = 20  # d-chunks
CH = D // NCH  # 500

FP8 = mybir.dt.float8e4
BF16 = mybir.dt.bfloat16
F32 = mybir.dt.float32
NP_FP8 = np.dtype(mybir.dt.np(FP8))
NP_BF16 = np.dtype(mybir.dt.np(BF16))

_CACHE = {}


def _build_program():
    nc = bacc.Bacc("TRN2", target_bir_lowering=False, debug=False, num_devices=N_CORES)

    table_p = nc.declare_dram_parameter("table", [128, KP, 2, D], FP8, isOutput=False)
    oh_p = nc.declare_dram_parameter("onehot", [128, B_LOC, KP, 2, T], FP8, isOutput=False)
    eb_p = nc.declare_dram_parameter("eb", [128, 4 * B_LOC], F32, isOutput=False)
    out_p = nc.declare_dram_parameter("sample", [B_LOC, NCH, CH], F32, isOutput=True)

    with TileContext(nc) as tc:
        with (
            tc.tile_pool(name="const", bufs=1) as cpool,
            tc.tile_pool(name="bund", bufs=B_LOC) as bpool,
            tc.tile_pool(name="work", bufs=3) as wpool,
            tc.tile_pool(name="gram", bufs=3) as gpool,
            tc.tile_pool(name="psA", bufs=7, space="PSUM") as psA_pool,
            tc.tile_pool(name="psB", bufs=1, space="PSUM") as psB_pool,
        ):
            table_sb = cpool.tile([128, KP, 2, D], FP8, tag="table")
            for kp in range(KP):
                nc.sync.dma_start(out=table_sb[:, kp, :, :], in_=table_p[:, kp, :, :])
            oh_sb = cpool.tile([128, B_LOC, KP, 2, T], FP8, tag="oh")
            nc.sync.dma_start(out=oh_sb[:], in_=oh_p[:])
            eb_sb = cpool.tile([128, 4 * B_LOC], F32, tag="eb")
            nc.sync.dma_start(out=eb_sb[:], in_=eb_p[:])

            # ---- Phase A: bundled[b] = onehot[b].T @ table ----
            bund = []
            for b in range(B_LOC):
                bund_b = bpool.tile([128, D], FP8, tag="bund")
                # groups of 7 chunks share a weight load per k-tile
                for g0 in range(0, NCH, 7):
                    chunks = range(g0, min(g0 + 7, NCH))
                    ps = {
                        c: psA_pool.tile([128, CH], F32, tag="psA", name=f"psA{c}")
                        for c in chunks
                    }
                    for kp in range(KP):
                        lhsT = oh_sb[:, b, kp, :, :]
                        for c in chunks:
                            nc.tensor.matmul(
                                ps[c][:],
                                lhsT,
                                table_sb[:, kp, :, c * CH : (c + 1) * CH],
                                start=(kp == 0),
                                stop=(kp == KP - 1),
                                perf_mode=mybir.MatmulPerfMode.DoubleRow,
                            )
                    for c in chunks:
                        nc.scalar.copy(
                            out=bund_b[:, c * CH : (c + 1) * CH], in_=ps[c][:]
                        )
                bund.append(bund_b)

            # ---- Phase B: ngram product + t'-reduce ----
            NT2 = T - 1  # 127 u rows
            NTP = T - N_GRAM + 1  # 125 gram rows
            for c in range(NCH):
                psB = psB_pool.tile([4, CH], F32, tag="psB")
                for b in range(B_LOC):
                    bd = bund[b]
                    base = c * CH - 2
                    # engine APs need 32-aligned partition starts, so the
                    # t+1 / t'+2 shifted operands are staged via DMA.
                    # sh1[p, j] = bd[p+1, (base+j)%D]
                    sh1 = wpool.tile([128, CH + 2], FP8, tag="sh1")
                    if c == 0:
                        nc.sync.dma_start(
                            out=sh1[:NT2, 0:2], in_=bd[1:T, D - 2 : D]
                        )
                        nc.sync.dma_start(out=sh1[:NT2, 2 : CH + 2], in_=bd[1:T, 0:CH])
                    else:
                        nc.sync.dma_start(
                            out=sh1[:NT2, :], in_=bd[1:T, base : base + CH + 2]
                        )
                    # u_t[t, j] = u[t, base+j] = bd[t, (base-1+j)%D] * sh1[t, j]
                    u_t = wpool.tile([128, CH + 2], BF16, tag="u")
                    if c == 0:
                        nc.vector.tensor_mul(
                            out=u_t[:NT2, 0:3],
                            in0=bd[:NT2, D - 3 : D],
                            in1=sh1[:NT2, 0:3],
                        )
                        nc.vector.tensor_mul(
                            out=u_t[:NT2, 3 : CH + 2],
                            in0=bd[:NT2, 0 : CH - 1],
                            in1=sh1[:NT2, 3 : CH + 2],
                        )
                    else:
                        nc.vector.tensor_mul(
                            out=u_t[:NT2, :],
                            in0=bd[:NT2, base - 1 : base + CH + 1],
                            in1=sh1[:NT2, :],
                        )
                    # ush[p, i] = u_t[p+2, i+2]
                    ush = wpool.tile([128, CH], BF16, tag="ush")
                    nc.sync.dma_start(
                        out=ush[:NTP, :], in_=u_t[2 : NTP + 2, 2 : CH + 2]
                    )
                    # gram[t', i] = u[t', i] * u[t'+2, i+2]  (i indexes chunk cols)
                    gram = gpool.tile([128, CH], F32, tag="gram")
                    nc.vector.tensor_mul(
                        out=gram[:NTP, :],
                        in0=u_t[:NTP, 0:CH],
                        in1=ush[:NTP, :],
                    )
                    # t'-reduce into row b of psB via selection column
                    nc.tensor.matmul(
                        psB[:],
                        eb_sb[:NTP, b * 4 : (b + 1) * 4],
                        gram[:NTP, :],
                        start=(b == 0),
                        stop=(b == B_LOC - 1),
                    )
                samp_sb = gpool.tile([4, CH], F32, tag="samp")
                nc.scalar.copy(out=samp_sb[:], in_=psB[:])
                nc.sync.dma_start(out=out_p[:, c, :], in_=samp_sb[:])

    nc.finalize()
    return nc


def _host_prep(x, level_hv, channel_hv):
    # Bit-exact replication of the jax fp32 quantization
    x = np.asarray(x, dtype=np.float32)
    t1 = x + np.float32(100.0)
    t2 = t1 / np.float32(200.0)
    t3 = t2 * np.float32(200.0)
    idx = np.clip(np.rint(t3), 0, NUM_LEVELS - 1).astype(np.int32)  # [B,T,C]

    one = np.float32(1.0)
    fp8_one = np.array([1.0], dtype=np.float32).astype(NP_FP8)[0]
    fp8_mone = np.array([-1.0], dtype=np.float32).astype(NP_FP8)[0]

    # folded table, fp8 bytes, padded to 13*128 rows, laid out [128, 13, D]
    prod = (level_hv[None, :, :] * channel_hv[:, None, :]).reshape(K_TOT, D)
    tab = np.zeros((K_PAD, D), dtype=NP_FP8)
    tab[:K_TOT] = np.where(prod > 0, fp8_one, fp8_mone)
    tab = np.ascontiguousarray(tab.reshape(KP, 2, 128, D).transpose(2, 0, 1, 3))

    # one-hot planes per batch: [B, 128, KT, T] fp8
    oh = np.zeros((B, K_PAD, T), dtype=NP_FP8)
    bb, tt, cc = np.meshgrid(
        np.arange(B), np.arange(T), np.arange(C), indexing="ij"
    )
    kk = cc * NUM_LEVELS + idx
    oh[bb.ravel(), kk.ravel(), tt.ravel()] = fp8_one
    oh = np.ascontiguousarray(
        oh.reshape(B, KP, 2, 128, T).transpose(0, 3, 1, 2, 4)
    )  # [B, 128, KP, 2, T]

    # e_b selection columns [128, 16] bf16: col b*4+m = 1 iff m==b, rows < 125
    eb = np.zeros((128, 4 * B_LOC), dtype=np.float32)
    for b in range(B_LOC):
        eb[: T - N_GRAM + 1, b * 4 + b] = one
    return idx, tab, oh, eb


def kernel(x, level_hv, channel_hv, centroid):
    if "nc" not in _CACHE:
        _CACHE["nc"] = _build_program()
    nc = _CACHE["nc"]

    idx, tab, oh, eb = _host_prep(x, level_hv, channel_hv)

    in_maps = []
    for core in range(N_CORES):
        bs = slice(core * B_LOC, (core + 1) * B_LOC)
        oh_core = np.ascontiguousarray(
            oh[bs].transpose(1, 0, 2, 3, 4)
        )  # [128, B_LOC, KP, 2, T]
        in_maps.append({"table": tab, "onehot": oh_core, "eb": eb})

    res = run_bass_kernel_spmd(nc, in_maps, list(range(N_CORES)))
    _CACHE["last_results"] = res

    sample = np.concatenate(
        [res.results[i]["sample"].reshape(B_LOC, D) for i in range(N_CORES)], axis=0
    )  # [32, 10000]
    sign = np.where(sample > 0, np.float32(1.0), np.float32(-1.0))
    return (sign @ np.asarray(centroid, dtype=np.float32).T).astype(np.float32)



# revision 4
# speedup vs baseline: 2.2235x; 2.2235x over previous
"""HDClassifier Trainium2 kernel (v2).

Math (per batch b):
  idx[t,c]   = clip(round((x+100)/200*200), 0, 200)
  bundled[t] = sum_c level_hv[idx[t,c]] * channel_hv[c]          # ints in [-8,8]
  gram[t',d] = prod_{i=0..3} bundled[t'+i, (d-(3-i)) mod D]      # |.| <= 4096
  sample[d]  = sum_{t'=0..124} gram[t',d]
  out        = sign(sample) @ centroid.T

Device strategy (8 cores, 4 batches each):
  - Host compacts the folded table per core: only the ~1350 (channel,level)
    keys actually used by the core's 4 batches are uploaded, padded to
    KP*256 rows (KP=6 expected) -> 6 DoubleRow k-passes instead of 7.
  - Rows carry a 3-col circular halo on the left (cols 9997..9999,0..9999),
    so the n-gram's circular d-shifts become plain free-dim offsets.
  - Phase A (chunk-major, table streamed): per 512-col chunk, per batch,
    KP fp8 DoubleRow matmuls accumulate onehot.T @ table in PSUM; Act
    drains to a per-batch fp8 bundled tile [128, 10003].
  - Phase B (per quarter, per batch): DMA-stage the partition-shifted
    bundled (sh1) and u (ush); DVE: u = bund*sh1 (fp16), gram = u*ush
    (fp16, values <= 4096 exact-ish in fp16).
  - t'-reduce: one fp16 matmul per (batch, chunk) with a ones-column
    lhsT view selecting row 20b+c of a single [80,512] f32 PSUM bank
    accumulated across all 80 matmuls -> one drain + one output DMA.
  - Host: sign + tiny [32,10000]@[10000,6] matmul.
"""

import sys

sys.path.insert(0, "/opt/trn_rl_repo")

import numpy as np

import concourse.bass as bass
import concourse.mybir as mybir
from concourse import bacc
from concourse.bass_utils import run_bass_kernel_spmd
from concourse.tile import TileContext

# Problem constants (hardcoded per contract)
NUM_LEVELS = 201
N_GRAM = 4
B, T, C, D, NUM_CLASSES = 32, 128, 8, 10000, 6
N_CORES = 8
B_LOC = B // N_CORES  # 4 batches per core
K_TOT = C * NUM_LEVELS  # 1608
HALO = N_GRAM - 1  # 3
DL = D + HALO  # 10003 local bundled width

CH = 512
NCH = 20  # ceil(10003/512) == ceil(10000/512) == 20
CHW_A = [min(CH, DL - CH * c) for c in range(NCH)]  # phase-A widths (last 275)
CHW_B = [min(CH, D - CH * c) for c in range(NCH)]  # out-chunk widths (last 272)
NQ = 4  # phase-B quarters (5 chunks each)
QW = [min(5 * CH, D - 5 * CH * q) for q in range(NQ)]  # 2560,2560,2560,2320

FP8 = mybir.dt.float8e4
FP16 = mybir.dt.float16
F32 = mybir.dt.float32
NP_FP8 = np.dtype(mybir.dt.np(FP8))
NP_FP16 = np.dtype(mybir.dt.np(FP16))

_CACHE = {}


def _build_program(kp):
    nc = bacc.Bacc("TRN2", target_bir_lowering=False, debug=False, num_devices=N_CORES)

    table_p = nc.declare_dram_parameter("table", [128, kp, 2, DL], FP8, isOutput=False)
    oh_p = nc.declare_dram_parameter("onehot", [128, kp, 2, B_LOC, T], FP8, isOutput=False)
    eb_p = nc.declare_dram_parameter("eb", [128, 2 * 80 + 1], FP16, isOutput=False)
    out_p = nc.declare_dram_parameter("sample", [80, CH], F32, isOutput=True)

    with TileContext(nc) as tc:
        with (
            tc.tile_pool(name="const", bufs=1) as cpool,
            tc.tile_pool(name="tab", bufs=4) as tpool,
            tc.tile_pool(name="bund", bufs=1) as bpool,
            tc.tile_pool(name="sh1", bufs=3) as shpool,
            tc.tile_pool(name="u1", bufs=3) as upool,
            tc.tile_pool(name="ush", bufs=3) as uspool,
            tc.tile_pool(name="gram", bufs=3) as gpool,
            tc.tile_pool(name="psA", bufs=6, space="PSUM") as psA_pool,
            tc.tile_pool(name="psB", bufs=1, space="PSUM") as psB_pool,
        ):
            oh_sb = cpool.tile([128, kp, 2, B_LOC, T], FP8, tag="oh")
            nc.sync.dma_start(out=oh_sb[:], in_=oh_p[:])
            eb_sb = cpool.tile([128, 2 * 80 + 1], FP16, tag="eb")
            nc.sync.dma_start(out=eb_sb[:], in_=eb_p[:])

            bund = [
                bpool.tile([128, DL], FP8, tag=f"bund{b}", name=f"bund{b}")
                for b in range(B_LOC)
            ]
            psBIG = psB_pool.tile([80, CH], F32, tag="psBIG")
            nred = [0]  # count of reduce matmuls emitted

            def phase_b_quarter(q):
                q0 = 5 * CH * q  # d-offset of quarter
                wq = QW[q]
                for b in range(B_LOC):
                    bd = bund[b]
                    # sh1[t, j] = bund[t+1, q0+j+1], j in [0, wq+2)
                    sh1 = shpool.tile([128, 5 * CH + 2], FP8, tag="sh1")
                    nc.sync.dma_start(
                        out=sh1[:127, 0 : wq + 2], in_=bd[1:128, q0 + 1 : q0 + wq + 3]
                    )
                    # u1[t, j] = bund[t, q0+j] * bund[t+1, q0+j+1]
                    u1 = upool.tile([128, 5 * CH + 2], FP16, tag="u1")
                    nc.vector.tensor_mul(
                        out=u1[:127, 0 : wq + 2],
                        in0=bd[:127, q0 : q0 + wq + 2],
                        in1=sh1[:127, 0 : wq + 2],
                    )
                    # ush[p, j] = u1[p+2, j+2]
                    ush = uspool.tile([128, 5 * CH], FP16, tag="ush")
                    nc.sync.dma_start(
                        out=ush[:125, 0:wq], in_=u1[2:127, 2 : wq + 2]
                    )
                    for l in range(5):
                        c = 5 * q + l
                        w = CHW_B[c]
                        off = CH * l
                        gram = gpool.tile([128, CH], FP16, tag="gram")
                        nc.vector.tensor_mul(
                            out=gram[:125, 0:w],
                            in0=u1[:125, off : off + w],
                            in1=ush[:125, off : off + w],
                        )
                        r = b * NCH + c
                        nc.tensor.matmul(
                            psBIG[:, 0:w],
                            eb_sb[:125, 80 - r : 160 - r],
                            gram[:125, 0:w],
                            start=(nred[0] == 0),
                            stop=(nred[0] == B_LOC * NCH - 1),
                        )
                        nred[0] += 1

            for c in range(NCH):
                w = CHW_A[c]
                c0 = CH * c
                tab = tpool.tile([128, kp, 2, CH], FP8, tag="tab")
                nc.sync.dma_start(out=tab[:, :, :, 0:w], in_=table_p[:, :, :, c0 : c0 + w])
                for b in range(B_LOC):
                    ps = psA_pool.tile([128, w], F32, tag="psA", name=f"psA{c}_{b}")
                    for k in range(kp):
                        nc.tensor.matmul(
                            ps[:],
                            oh_sb[:, k, :, b, :],
                            tab[:, k, :, 0:w],
                            start=(k == 0),
                            stop=(k == kp - 1),
                            perf_mode=mybir.MatmulPerfMode.DoubleRow,
                        )
                    nc.scalar.copy(out=bund[b][:, c0 : c0 + w], in_=ps[:])
                # quarter q reads bund cols up to 2560q+2562, so it must wait
                # for chunk 5(q+1) (the 2-col overhang); quarter 3 ends at
                # col 10002 which the last chunk covers.
                if c in (5, 10, 15):
                    phase_b_quarter(c // 5 - 1)
                elif c == NCH - 1:
                    phase_b_quarter(NQ - 1)

            samp = cpool.tile([80, CH], F32, tag="samp")
            nc.scalar.copy(out=samp[:], in_=psBIG[:])
            nc.sync.dma_start(out=out_p[:], in_=samp[:])

    nc.finalize()
    return nc


def _host_prep(x, level_hv, channel_hv):
    # Bit-exact replication of the jax fp32 quantization
    x = np.asarray(x, dtype=np.float32)
    t1 = x + np.float32(100.0)
    t2 = t1 / np.float32(200.0)
    t3 = t2 * np.float32(200.0)
    idx = np.clip(np.rint(t3), 0, NUM_LEVELS - 1).astype(np.int32)  # [B,T,C]

    fp8_one = np.array([1.0], dtype=np.float32).astype(NP_FP8)[0]
    fp8_mone = np.array([-1.0], dtype=np.float32).astype(NP_FP8)[0]

    # folded +-1 table as fp8 bytes [1608, D]
    prod = (level_hv[None, :, :] * channel_hv[:, None, :]).reshape(K_TOT, D)
    F = np.where(prod > 0, fp8_one, fp8_mone)

    kk = np.arange(C, dtype=np.int32)[None, None, :] * NUM_LEVELS + idx  # [B,T,C]

    cores = []
    kp_max = 1
    for core in range(N_CORES):
        kk_c = kk[core * B_LOC : (core + 1) * B_LOC]  # [B_LOC, T, C]
        keys = np.unique(kk_c)
        n_k = len(keys)
        kp_c = -(-n_k // 256)
        kp_max = max(kp_max, kp_c)
        cores.append((kk_c, keys, n_k))

    kp = kp_max
    kpad = kp * 256
    in_maps = []
    eb = np.zeros((128, 2 * 80 + 1), dtype=NP_FP16)
    eb[: T - N_GRAM + 1, 80] = np.float16(1.0)
    for kk_c, keys, n_k in cores:
        inv = np.zeros(K_TOT, dtype=np.int32)
        inv[keys] = np.arange(n_k, dtype=np.int32)
        slots = inv[kk_c]  # [B_LOC, T, C]

        tabc = np.zeros((kpad, DL), dtype=NP_FP8)
        tabc[:n_k, HALO:] = F[keys]
        tabc[:n_k, :HALO] = F[keys][:, D - HALO :]
        table_up = np.ascontiguousarray(
            tabc.reshape(kp, 2, 128, DL).transpose(2, 0, 1, 3)
        )  # [128, kp, 2, DL]

        oh = np.zeros((B_LOC, kpad, T), dtype=NP_FP8)
        bb, tt, cc = np.meshgrid(
            np.arange(B_LOC), np.arange(T), np.arange(C), indexing="ij"
        )
        oh[bb.ravel(), slots.ravel(), tt.ravel()] = fp8_one
        oh_up = np.ascontiguousarray(
            oh.reshape(B_LOC, kp, 2, 128, T).transpose(3, 1, 2, 0, 4)
        )  # [128, kp, 2, B_LOC, T]

        in_maps.append({"table": table_up, "onehot": oh_up, "eb": eb})
    return kp, in_maps


def kernel(x, level_hv, channel_hv, centroid):
    kp, in_maps = _host_prep(x, level_hv, channel_hv)
    if kp not in _CACHE:
        _CACHE[kp] = _build_program(kp)
    nc = _CACHE[kp]

    res = run_bass_kernel_spmd(nc, in_maps, list(range(N_CORES)))
    _CACHE["last_results"] = res
    _CACHE["nc"] = nc

    sample = np.empty((B, D), dtype=np.float32)
    for core in range(N_CORES):
        arr = res.results[core]["sample"]  # [80, 512]
        for b in range(B_LOC):
            row = arr[b * NCH : (b + 1) * NCH]  # [20, 512]
            for c in range(NCH):
                w = CHW_B[c]
                sample[core * B_LOC + b, CH * c : CH * c + w] = row[c, :w]
    sign = np.where(sample > 0, np.float32(1.0), np.float32(-1.0))
    return (sign @ np.asarray(centroid, dtype=np.float32).T).astype(np.float32)
